# revision 13
# baseline (speedup 1.0000x reference)
"""Trainium2 Bass kernel for nn_DglAggregator (two-stage GNN message passing).

Strategy: edges are sorted by destination and bucketed into 128-node
destination windows; windows are sharded contiguously across the 8 cores
(each core owns a contiguous slice of destination nodes, so no cross-core
reduction is needed).  Per 128-edge tile, destination-selection matrices
(one-hot by window-local dst) turn segment softmax / segment sum into PE
matmuls that accumulate in PSUM; source-side feature rows are fetched with
batched indirect DMA gathers.  Two launches: stage 1 produces the per-item
aggregated features ft, the host concatenates the slices, stage 2 consumes
the full ft table.

  stage 1:  e  = leakyrelu(sum_d h_n[src]*h_n[dst]*p_w)   per edge
            ee = exp(e)              (no max-subtraction; e is O(1))
            ft[n] = (sum_e ee*h_n[src]) / max(sum_e ee, eps-free guard)
  stage 2:  eft = ft[src_a];  e2 = tanh([eft | h_p] @ q_w.T)
            coef = sum_d e2 * h_t[dst_a]
            out[n] = sum_e coef * eft
"""

import numpy as np

import concourse.bass as bass
import concourse.tile as tile
import concourse.mybir as mybir
from concourse import bacc
from concourse.bass_utils import run_bass_kernel_spmd

P = 128
D = 128
N_ITEMS = 50000
N_TARGETS = 50000
E_INT = 800000
E_AGG = 800000
N_CORES = 8
ALPHA = 0.2

F32 = mybir.dt.float32
I32 = mybir.dt.int32
I16 = mybir.dt.int16
Alu = mybir.AluOpType
Act = mybir.ActivationFunctionType

# test.py reads the BassKernelResults of the last kernel() call from here
# (exec_time_ns is populated when BASS_TRACE=1 is set in the environment).
LAST_RESULTS = []

_PROGRAM_CACHE = {}


# ----------------------------------------------------------------------------
# device programs
# ----------------------------------------------------------------------------

def build_stage1(TBL1, NLOC, NW, T1):
    nc = bacc.Bacc("TRN2", target_bir_lowering=False, debug=False)
    NI = T1 * P            # gather slots per window
    IW = NI // 16          # wrapped-index free dim

    src_tab = nc.dram_tensor("src_tab", [NW, TBL1, D], F32, kind="ExternalInput")
    g_loc = nc.dram_tensor("g_loc", [NLOC, D], F32, kind="ExternalInput")
    sidx = nc.dram_tensor("sidx", [NW, P, IW], I16, kind="ExternalInput")
    didx = nc.dram_tensor("didx", [NW, P, IW], I16, kind="ExternalInput")
    dstl = nc.dram_tensor("dstl", [NW, P, T1], F32, kind="ExternalInput")
    iota_f = nc.dram_tensor("iota_f", [P, P], F32, kind="ExternalInput")
    ft_out = nc.dram_tensor("ft_out", [NLOC, D], F32, kind="ExternalOutput")

    with tile.TileContext(nc) as tc:
        with (
            tc.tile_pool(name="const", bufs=1) as cpool,
            tc.tile_pool(name="idx", bufs=2) as ipool,
            tc.tile_pool(name="gath", bufs=2) as gpool,
            tc.tile_pool(name="work", bufs=4) as wpool,
            tc.tile_pool(name="outp", bufs=2) as opool,
            tc.tile_pool(name="psum", bufs=2, space="PSUM") as ppool,
        ):
            iota_sb = cpool.tile([P, P], F32, tag="iota")
            nc.sync.dma_start(out=iota_sb[:], in_=iota_f[:])
            ni_reg = nc.gpsimd.to_reg(NI)

            for w in range(NW):
                sidx_sb = ipool.tile([P, IW], I16, tag="sidx")
                nc.sync.dma_start(out=sidx_sb[:], in_=sidx[w])
                didx_sb = ipool.tile([P, IW], I16, tag="didx")
                nc.sync.dma_start(out=didx_sb[:], in_=didx[w])
                dstl_sb = ipool.tile([P, T1], F32, tag="dstl")
                nc.sync.dma_start(out=dstl_sb[:], in_=dstl[w])

                hsrc = gpool.tile([P, T1, D], F32, tag="hsrc")
                nc.gpsimd.dma_gather(hsrc[:], src_tab[w], sidx_sb[:], NI, ni_reg, D, single_packet=False)
                gdst = gpool.tile([P, T1, D], F32, tag="gdst")
                nc.gpsimd.dma_gather(gdst[:], g_loc[:], didx_sb[:], NI, ni_reg, D, single_packet=False)

                U = ppool.tile([P, 129], F32, tag="U", space="PSUM")
                for t in range(T1):
                    hs = hsrc[:, t, :]
                    gd = gdst[:, t, :]
                    scratch = wpool.tile([P, D], F32, tag="scratch")
                    e_raw = wpool.tile([P, 1], F32, tag="eraw")
                    nc.vector.tensor_tensor(out=scratch[:], in0=hs, in1=gd,
                                            op=Alu.mult)
                    nc.vector.tensor_reduce(out=e_raw[:], in_=scratch[:],
                                            axis=mybir.AxisListType.X,
                                            op=Alu.add)
                    e_s = wpool.tile([P, 1], F32, tag="es")
                    nc.vector.tensor_scalar(out=e_s[:], in0=e_raw[:],
                                            scalar1=ALPHA, scalar2=None,
                                            op0=Alu.mult)
                    ee = wpool.tile([P, 1], F32, tag="ee")
                    nc.vector.tensor_tensor(out=ee[:], in0=e_raw[:],
                                            in1=e_s[:], op=Alu.max)
                    nc.scalar.activation(ee[:], ee[:], Act.Exp)
                    msgee = wpool.tile([P, 129], F32, tag="msgee")
                    nc.scalar.activation(msgee[:, :D], hs, Act.Copy, scale=ee[:])
                    nc.vector.tensor_copy(msgee[:, D:129], ee[:])
                    S = wpool.tile([P, P], F32, tag="S")
                    nc.vector.tensor_scalar(
                        out=S[:], in0=iota_sb[:], scalar1=dstl_sb[:, t:t + 1],
                        scalar2=None, op0=Alu.is_equal)
                    nc.tensor.matmul(U[:, :129], lhsT=S[:], rhs=msgee[:],
                                     start=(t == 0), stop=(t == T1 - 1))

                dz = wpool.tile([P, 1], F32, tag="dz")
                nc.vector.tensor_scalar(out=dz[:], in0=U[:, D:129], scalar1=0.0,
                                        scalar2=None, op0=Alu.is_equal)
                den = wpool.tile([P, 1], F32, tag="den")
                nc.vector.tensor_scalar(out=den[:], in0=U[:, D:129],
                                        scalar1=dz[:], scalar2=None, op0=Alu.add)
                r = wpool.tile([P, 1], F32, tag="r")
                nc.vector.reciprocal(r[:], den[:])
                ftw = opool.tile([P, D], F32, tag="ftw")
                nc.scalar.activation(ftw[:], U[:, :D], Act.Copy, scale=r[:])
                nc.sync.dma_start(out=ft_out[w * P:(w + 1) * P, :], in_=ftw[:])
    return nc


def build_stage2(TBL2, NW, T2):
    nc = bacc.Bacc("TRN2", target_bir_lowering=False, debug=False)
    S2 = NW * T2 * P
    NI = T2 * P
    IW = NI // 16

    eft_tab = nc.dram_tensor("eft_tab", [NW, TBL2, D], F32, kind="ExternalInput")
    aidx = nc.dram_tensor("aidx", [NW, P, IW], I16, kind="ExternalInput")
    hp_t = nc.dram_tensor("hp_t", [D, S2], F32, kind="ExternalInput")
    h_t = nc.dram_tensor("h_t", [NW * P, D], F32, kind="ExternalInput")
    qw_t = nc.dram_tensor("qw_t", [2 * D, D], F32, kind="ExternalInput")
    dstl = nc.dram_tensor("dstl", [NW, P, T2], F32, kind="ExternalInput")
    dstl_r = nc.dram_tensor("dstl_r", [NW, T2 * P], F32, kind="ExternalInput")
    iota_f = nc.dram_tensor("iota_f", [P, P], F32, kind="ExternalInput")
    iota_p = nc.dram_tensor("iota_p", [P, 1], F32, kind="ExternalInput")
    ones_r = nc.dram_tensor("ones_r", [1, P], F32, kind="ExternalInput")
    ones_c = nc.dram_tensor("ones_c", [P, 1], F32, kind="ExternalInput")
    ident = nc.dram_tensor("ident", [P, P], F32, kind="ExternalInput")
    out = nc.dram_tensor("out", [NW * P, D], F32, kind="ExternalOutput")

    with tile.TileContext(nc) as tc:
        with (
            tc.tile_pool(name="const", bufs=1) as cpool,
            tc.tile_pool(name="idx", bufs=2) as ipool,
            tc.tile_pool(name="gath", bufs=2) as gpool,
            tc.tile_pool(name="hpp", bufs=2) as hpool,
            tc.tile_pool(name="work", bufs=4) as wpool,
            tc.tile_pool(name="outp", bufs=2) as opool,
            tc.tile_pool(name="psA", bufs=4, space="PSUM") as ppA,
            tc.tile_pool(name="psO", bufs=2, space="PSUM") as ppO,
        ):
            iota_sb = cpool.tile([P, P], F32, tag="iota")
            nc.sync.dma_start(out=iota_sb[:], in_=iota_f[:])
            iop_sb = cpool.tile([P, 1], F32, tag="iop")
            nc.sync.dma_start(out=iop_sb[:], in_=iota_p[:])
            ones_r_sb = cpool.tile([1, P], F32, tag="onesr")
            nc.sync.dma_start(out=ones_r_sb[:], in_=ones_r[:])
            ones_c_sb = cpool.tile([P, 1], F32, tag="onesc")
            nc.sync.dma_start(out=ones_c_sb[:], in_=ones_c[:])
            id_sb = cpool.tile([P, P], F32, tag="ident")
            nc.sync.dma_start(out=id_sb[:], in_=ident[:])
            qw0_sb = cpool.tile([D, D], F32, tag="qw0")
            nc.sync.dma_start(out=qw0_sb[:], in_=qw_t[0:D, :])
            qw1_sb = cpool.tile([D, D], F32, tag="qw1")
            nc.sync.dma_start(out=qw1_sb[:], in_=qw_t[D:2 * D, :])
            ni_reg = nc.gpsimd.to_reg(NI)

            for w in range(NW):
                aidx_sb = ipool.tile([P, IW], I16, tag="aidx")
                nc.sync.dma_start(out=aidx_sb[:], in_=aidx[w])
                dstl_sb = ipool.tile([P, T2], F32, tag="dstl")
                nc.sync.dma_start(out=dstl_sb[:], in_=dstl[w])
                dstlr_sb = ipool.tile([1, T2 * P], F32, tag="dstlr")
                nc.sync.dma_start(out=dstlr_sb[:], in_=dstl_r[w:w + 1, :])
                htw_sb = ipool.tile([P, D], F32, tag="htw")
                nc.sync.dma_start(out=htw_sb[:], in_=h_t[w * P:(w + 1) * P, :])

                eftw = gpool.tile([P, T2, D], F32, tag="eft")
                nc.gpsimd.dma_gather(eftw[:], eft_tab[w], aidx_sb[:], NI, ni_reg, D, single_packet=False)
                hptw = hpool.tile([D, T2 * P], F32, tag="hpt")
                nc.sync.dma_start(
                    out=hptw[:], in_=hp_t[:, w * T2 * P:(w + 1) * T2 * P])

                ow = ppO.tile([P, D], F32, tag="ow", space="PSUM")
                for t in range(T2):
                    eft = eftw[:, t, :]
                    eftT_ps = ppA.tile([P, P], F32, tag="ps", space="PSUM")
                    nc.tensor.transpose(out=eftT_ps[:], in_=eft, identity=id_sb[:])
                    eftT = wpool.tile([P, P], F32, tag="eftT")
                    nc.scalar.copy(eftT[:], eftT_ps[:])
                    e2_ps = ppA.tile([P, P], F32, tag="ps", space="PSUM")
                    nc.tensor.matmul(e2_ps[:], lhsT=qw0_sb[:], rhs=eftT[:],
                                     start=True, stop=False)
                    nc.tensor.matmul(e2_ps[:], lhsT=qw1_sb[:],
                                     rhs=hptw[:, t * P:(t + 1) * P],
                                     start=False, stop=True)
                    e2T = wpool.tile([P, P], F32, tag="e2T")
                    nc.scalar.activation(e2T[:], e2_ps[:], Act.Tanh)
                    dstb_ps = ppA.tile([P, P], F32, tag="ps", space="PSUM")
                    nc.tensor.matmul(dstb_ps[:], lhsT=ones_r_sb[:],
                                     rhs=dstlr_sb[:, t * P:(t + 1) * P],
                                     start=True, stop=True)
                    ST = wpool.tile([P, P], F32, tag="ST")
                    nc.vector.tensor_scalar(out=ST[:], in0=dstb_ps[:],
                                            scalar1=iop_sb[:], scalar2=None,
                                            op0=Alu.is_equal)
                    htg_ps = ppA.tile([P, P], F32, tag="ps", space="PSUM")
                    nc.tensor.matmul(htg_ps[:], lhsT=htw_sb[:], rhs=ST[:],
                                     start=True, stop=True)
                    prod = wpool.tile([P, P], F32, tag="prod")
                    nc.vector.tensor_tensor(out=prod[:], in0=e2T[:],
                                            in1=htg_ps[:], op=Alu.mult)
                    coef_ps = ppA.tile([P, 1], F32, tag="ps", space="PSUM")
                    nc.tensor.matmul(coef_ps[:], lhsT=prod[:], rhs=ones_c_sb[:],
                                     start=True, stop=True)
                    coef = wpool.tile([P, 1], F32, tag="coefs")
                    nc.scalar.copy(coef[:], coef_ps[:])
                    Sc = wpool.tile([P, P], F32, tag="Sc")
                    nc.vector.tensor_scalar(
                        out=Sc[:], in0=iota_sb[:], scalar1=dstl_sb[:, t:t + 1],
                        scalar2=coef[:], op0=Alu.is_equal, op1=Alu.mult)
                    nc.tensor.matmul(ow[:], lhsT=Sc[:], rhs=eft,
                                     start=(t == 0), stop=(t == T2 - 1))

                osb = opool.tile([P, D], F32, tag="osb")
                nc.vector.tensor_copy(osb[:], ow[:])
                nc.sync.dma_start(out=out[w * P:(w + 1) * P, :], in_=osb[:])
    return nc


# ----------------------------------------------------------------------------
# host-side prep
# ----------------------------------------------------------------------------

def _bucket_edges(dst, n_cores, nw_pc):
    order = np.argsort(dst, kind="stable")
    dsts = dst[order]
    win = dsts // P
    counts = np.bincount(win, minlength=nw_pc * n_cores)
    starts = np.concatenate([[0], np.cumsum(counts)])
    rank = np.arange(dst.shape[0]) - starts[win]
    return order, win, rank, counts


def _wrap_idx(idx_flat):
    """[NI] int -> wrapped [128, NI//16] int16 (16-partition wrap, x8 replicas)."""
    w = np.ascontiguousarray(idx_flat.reshape(-1, 16).T).astype(np.int16)
    return np.ascontiguousarray(np.tile(w, (8, 1)))


def prep_stage1(h_n, p_w, src_i, dst_i, n_cores):
    N = h_n.shape[0]
    nw_total = (N + P - 1) // P
    nw_pc = (nw_total + n_cores - 1) // n_cores
    NLOC = nw_pc * P

    order, win, rank, counts = _bucket_edges(dst_i, n_cores, nw_pc)
    T1 = int(np.ceil(counts.max() / P))
    NI = T1 * P
    srcs = src_i[order]
    dsts = dst_i[order]

    core = win // nw_pc
    wl = win % nw_pc
    t = rank // P
    p = rank % P

    src_slot = np.zeros((n_cores, nw_pc, NI), np.int64)
    dst_slot = np.zeros((n_cores, nw_pc, NI), np.int64)
    dstl = np.full((n_cores, nw_pc, P, T1), -1.0, np.float32)
    src_slot[core, wl, rank] = srcs
    dst_slot[core, wl, rank] = dsts - core.astype(np.int64) * NLOC
    dstl[core, wl, p, t] = (dsts - win * P).astype(np.float32)

    # compact per-window source tables (int16 gather index limit)
    uniqs = [[None] * nw_pc for _ in range(n_cores)]
    invs = [[None] * nw_pc for _ in range(n_cores)]
    TBL1 = 0
    for c in range(n_cores):
        for w in range(nw_pc):
            u, inv = np.unique(src_slot[c, w], return_inverse=True)
            uniqs[c][w] = u
            invs[c][w] = inv
            TBL1 = max(TBL1, len(u))

    G = (h_n * p_w[0][None, :]).astype(np.float32)
    g_pad = np.zeros((n_cores * NLOC, D), np.float32)
    g_pad[:N] = G

    iota_f = np.tile(np.arange(P, dtype=np.float32)[None, :], (P, 1))
    h_n32 = np.asarray(h_n, dtype=np.float32)

    in_maps = []
    for c in range(n_cores):
        src_tab = np.zeros((nw_pc, TBL1, D), np.float32)
        sidx = np.empty((nw_pc, P, NI // 16), np.int16)
        didx = np.empty((nw_pc, P, NI // 16), np.int16)
        for w in range(nw_pc):
            u = uniqs[c][w]
            src_tab[w, :len(u)] = h_n32[u]
            sidx[w] = _wrap_idx(invs[c][w])
            didx[w] = _wrap_idx(dst_slot[c, w])
        in_maps.append({
            "src_tab": src_tab,
            "g_loc": np.ascontiguousarray(g_pad[c * NLOC:(c + 1) * NLOC]),
            "sidx": sidx,
            "didx": didx,
            "dstl": np.ascontiguousarray(dstl[c]),
            "iota_f": iota_f,
        })
    return in_maps, dict(nw_pc=nw_pc, T1=T1, NLOC=NLOC, nw_total=nw_total,
                         TBL1=TBL1)


def prep_stage2(h_p, h_t, q_w, src_a, dst_a, ft_full, n_cores):
    NT = h_t.shape[0]
    nw_total = (NT + P - 1) // P
    nw_pc = (nw_total + n_cores - 1) // n_cores
    NLOC = nw_pc * P

    order, win, rank, counts = _bucket_edges(dst_a, n_cores, nw_pc)
    T2 = int(np.ceil(counts.max() / P))
    srcs = src_a[order]
    dsts = dst_a[order]

    core = win // nw_pc
    wl = win % nw_pc
    t = rank // P
    p = rank % P

    NI = T2 * P
    src_slot = np.zeros((n_cores, nw_pc, NI), np.int64)
    dstl = np.full((n_cores, nw_pc, P, T2), -1.0, np.float32)
    slot_orig = np.full((n_cores, nw_pc, T2, P), -1, np.int64)
    src_slot[core, wl, rank] = srcs
    dstl[core, wl, p, t] = (dsts - win * P).astype(np.float32)
    slot_orig[core, wl, t, p] = order

    uniqs = [[None] * nw_pc for _ in range(n_cores)]
    invs = [[None] * nw_pc for _ in range(n_cores)]
    TBL2 = 0
    for c in range(n_cores):
        for w in range(nw_pc):
            u, inv = np.unique(src_slot[c, w], return_inverse=True)
            uniqs[c][w] = u
            invs[c][w] = inv
            TBL2 = max(TBL2, len(u))

    dstl_r = np.ascontiguousarray(
        dstl.transpose(0, 1, 3, 2)).reshape(n_cores, nw_pc, T2 * P)

    ht_pad = np.zeros((n_cores * NLOC, D), np.float32)
    ht_pad[:NT] = h_t

    iota_f = np.tile(np.arange(P, dtype=np.float32)[None, :], (P, 1))
    iota_p = np.arange(P, dtype=np.float32)[:, None].copy()
    ones_r = np.ones((1, P), np.float32)
    ones_c = np.ones((P, 1), np.float32)
    ident = np.eye(P, dtype=np.float32)
    qw_t = np.ascontiguousarray(q_w.T, dtype=np.float32)

    ft32 = np.asarray(ft_full, dtype=np.float32)
    in_maps = []
    for c in range(n_cores):
        so = slot_orig[c].reshape(-1)
        hp_slots = np.zeros((so.shape[0], D), np.float32)
        valid = so >= 0
        hp_slots[valid] = h_p[so[valid]]
        hp_t = np.ascontiguousarray(hp_slots.T)
        eft_tab = np.zeros((nw_pc, TBL2, D), np.float32)
        aidx = np.empty((nw_pc, P, NI // 16), np.int16)
        for w in range(nw_pc):
            u = uniqs[c][w]
            eft_tab[w, :len(u)] = ft32[u]
            aidx[w] = _wrap_idx(invs[c][w])
        in_maps.append({
            "eft_tab": eft_tab,
            "aidx": aidx,
            "hp_t": hp_t,
            "h_t": np.ascontiguousarray(ht_pad[c * NLOC:(c + 1) * NLOC]),
            "qw_t": qw_t,
            "dstl": np.ascontiguousarray(dstl[c]),
            "dstl_r": np.ascontiguousarray(dstl_r[c]),
            "iota_f": iota_f,
            "iota_p": iota_p,
            "ones_r": ones_r,
            "ones_c": ones_c,
            "ident": ident,
        })
    return in_maps, dict(nw_pc=nw_pc, T2=T2, NLOC=NLOC, nw_total=nw_total,
                         TBL2=TBL2)


# ----------------------------------------------------------------------------
# entry point
# ----------------------------------------------------------------------------

def _get_program(key, builder, *args):
    prog = _PROGRAM_CACHE.get(key)
    if prog is None:
        prog = builder(*args)
        prog.compile()
        _PROGRAM_CACHE[key] = prog
    return prog


def kernel(h_n, h_p, h_t, p_w, q_w, src_i, dst_i, src_a, dst_a):
    h_n = np.asarray(h_n, dtype=np.float32)
    h_p = np.asarray(h_p, dtype=np.float32)
    h_t = np.asarray(h_t, dtype=np.float32)
    p_w = np.asarray(p_w, dtype=np.float32)
    q_w = np.asarray(q_w, dtype=np.float32)
    src_i = np.asarray(src_i, dtype=np.int64)
    dst_i = np.asarray(dst_i, dtype=np.int64)
    src_a = np.asarray(src_a, dtype=np.int64)
    dst_a = np.asarray(dst_a, dtype=np.int64)

    LAST_RESULTS.clear()
    core_ids = list(range(N_CORES))

    # ---- stage 1 ----
    in_maps1, m1 = prep_stage1(h_n, p_w, src_i, dst_i, N_CORES)
    nc1 = _get_program(("s1", m1["TBL1"], m1["NLOC"], m1["nw_pc"], m1["T1"]),
                       build_stage1, m1["TBL1"], m1["NLOC"], m1["nw_pc"],
                       m1["T1"])
    res1 = run_bass_kernel_spmd(nc1, in_maps1, core_ids)
    LAST_RESULTS.append(res1)
    ft_full = np.concatenate(
        [res1.results[c]["ft_out"] for c in range(N_CORES)],
        axis=0)[:h_n.shape[0]]

    # ---- stage 2 ----
    in_maps2, m2 = prep_stage2(h_p, h_t, q_w, src_a, dst_a, ft_full, N_CORES)
    nc2 = _get_program(("s2", m2["TBL2"], m2["nw_pc"], m2["T2"]),
                       build_stage2, m2["TBL2"], m2["nw_pc"], m2["T2"])
    res2 = run_bass_kernel_spmd(nc2, in_maps2, core_ids)
    LAST_RESULTS.append(res2)
    out = np.concatenate(
        [res2.results[c]["out"] for c in range(N_CORES)],
        axis=0)[:h_t.shape[0]]
    return np.ascontiguousarray(out, dtype=np.float32)
